# revision 3
# baseline (speedup 1.0000x reference)
"""Trainium2 Bass kernel: binarized (XNOR/ReActNet-style) ResNet BasicBlock.

Computes, for x:[64,64,56,56] f32 and small per-channel parameters:

    out = PReLU_a(BN(conv3x3(sign(x + b0), scale * sign(w))) + x + b1) + b2

Distribution: data-parallel over the batch dim, 8 images per NeuronCore on
8 cores.  Per core, images (i, i+4) share the SBUF partition dim: channels
of the first image on partitions 0-63, channels of the second on 64-127.

Math folding (host side, all tiny tensors):
  - binarized weights sign(w) are pre-scaled by A_m = mean|w|_m * gamma_m /
    sqrt(var_m + eps)  (the BN multiplier), so PSUM holds BN-scaled conv.
    Products are +-A_m exactly, accumulated in fp32 PSUM -> only error is
    bf16 rounding of A_m itself (~2^-9 relative).
  - C_m = beta - mean*inv + bias1 + bias2 is applied as the ScalarE
    activation bias while reading PSUM.
  - residual +x is accumulated into PSUM with an identity matmul (bf16 x).
  - PReLU + bias2: y = max(t, a*t + d), d = bias2*(1-a), valid for a<=1;
    one fused DVE scalar_tensor_tensor when d==0, general 3-op path else.

On-chip layout: zero-padded 58x58 bf16 planes so each 3x3 tap is a single
contiguous 464-element rhs slice; output processed in 8-row slices whose
DRAM footprint is 448 contiguous floats per channel (1792B DMA bursts).
"""

import sys

if "/opt/trn_rl_repo" not in sys.path:
    sys.path.insert(0, "/opt/trn_rl_repo")

import numpy as np

import concourse.bass as bass
import concourse.bacc as bacc
import concourse.mybir as mybir
from concourse.tile import TileContext
from concourse.bass_utils import run_bass_kernel_spmd

AF = mybir.ActivationFunctionType
ALU = mybir.AluOpType
DT = mybir.dt

B, C, H, W = 64, 64, 56, 56
NCORES = 8
BPC = B // NCORES          # images per core
NPAIR = BPC // 2           # image pairs per core (img pr <-> partitions 0-63, img pr+NPAIR <-> 64-127)
HP, WP = H + 2, W + 2      # zero-padded plane 58x58
IMG = HP * WP              # 3364 elements per padded plane
RB = 8                     # output rows per slice
NSL = H // RB              # 7 slices per image
NT = RB * WP               # 464: matmul free size (contiguous in padded space)
NI = RB * W                # 448: interior (valid) elements per slice
BN_EPS = 1e-5

_NC_CACHE = {}


def _build(fast_prelu: bool):
    nc = bacc.Bacc("TRN2", target_bir_lowering=False, debug=False)
    x_ext = nc.declare_dram_parameter("x", [BPC, C, H, W], DT.float32, isOutput=False)
    w_ext = nc.declare_dram_parameter("wts", [128, 10 * 64], DT.bfloat16, isOutput=False)
    c_ext = nc.declare_dram_parameter("cst", [128, 8], DT.float32, isOutput=False)
    o_ext = nc.declare_dram_parameter("out", [BPC, C, H, W], DT.float32, isOutput=True)

    xg = x_ext.ap()
    og = o_ext.ap()

    with TileContext(nc) as tc:
        with tc.tile_pool(name="persist", bufs=1) as perst, \
             tc.tile_pool(name="work", bufs=4) as work, \
             tc.tile_pool(name="psum", bufs=4, space="PSUM") as ppool:

            wts = perst.tile([128, 10 * 64], DT.bfloat16)
            nc.sync.dma_start(out=wts, in_=w_ext.ap())
            cst = perst.tile([128, 8], DT.float32)
            nc.sync.dma_start(out=cst, in_=c_ext.ap())
            c_ap = cst[:, 0:1]    # beta - mean*inv + bias1 + bias2
            a_ap = cst[:, 1:2]    # PReLU alpha
            d_ap = cst[:, 2:3]    # bias2 * (1 - alpha)
            b0_ap = cst[:, 3:4]   # bias0

            xf = perst.tile([128, NPAIR * IMG], DT.float32)
            act = perst.tile([128, NPAIR * IMG], DT.bfloat16)

            # zero the padding ring of every plane in both buffers
            for pr in range(NPAIR):
                base = pr * IMG
                for buf in (xf, act):
                    v = buf[:, base:base + IMG].rearrange("p (h w) -> p h w", w=WP)
                    nc.vector.memset(v[:, 0:1, :], 0.0)
                    nc.vector.memset(v[:, HP - 1:HP, :], 0.0)
                    nc.vector.memset(v[:, 1:HP - 1, 0:1], 0.0)
                    nc.vector.memset(v[:, 1:HP - 1, WP - 1:WP], 0.0)

            def emit_loads(pr):
                base = pr * IMG
                ia, ib = pr, pr + NPAIR
                for rc in range(NSL):
                    h0 = rc * RB
                    dst = xf[:, base:base + IMG].rearrange(
                        "p (h w) -> p h w", w=WP)[:, 1 + h0:1 + h0 + RB, 1:1 + W]
                    nc.sync.dma_start(out=dst[0:64], in_=xg[ia, :, h0:h0 + RB, :])
                    nc.sync.dma_start(out=dst[64:128], in_=xg[ib, :, h0:h0 + RB, :])
                    adst = act[:, base:base + IMG].rearrange(
                        "p (h w) -> p h w", w=WP)[:, 1 + h0:1 + h0 + RB, 1:1 + W]
                    nc.scalar.activation(out=adst, in_=dst, func=AF.Sign,
                                         bias=b0_ap, scale=1.0)

            def emit_compute(pr):
                base = pr * IMG
                ia, ib = pr, pr + NPAIR
                for rc in range(NSL):
                    odd = (pr * NSL + rc) % 2
                    h0 = rc * RB
                    off0 = base + (h0 + 1) * WP

                    ps = ppool.tile([128, NT], DT.float32, tag="ps")
                    # bf16 copy of the x slice for the identity-matmul residual
                    xb = work.tile([128, NT], DT.bfloat16, tag="xb")
                    nc.vector.tensor_copy(out=xb, in_=xf[:, off0:off0 + NT])

                    # center tap first: it is never range-trimmed, so start=True
                    # clears the whole bank before the trimmed corner taps land
                    for t in (4, 0, 1, 2, 3, 5, 6, 7, 8):
                        dh, dw = t // 3 - 1, t % 3 - 1
                        off = base + (h0 + dh + 1) * WP + dw
                        s0 = 1 if (rc == 0 and dh == -1 and dw == -1) else 0
                        s1 = NT - 1 if (rc == NSL - 1 and dh == 1 and dw == 1) else NT
                        la = wts[0:64, t * 64:(t + 1) * 64]
                        lb = wts[64:128, t * 64:(t + 1) * 64]
                        ra = act[0:64, off + s0:off + s1]
                        rb = act[64:128, off + s0:off + s1]
                        pa = ps[64:128, s0:s1] if odd else ps[0:64, s0:s1]
                        pb = ps[0:64, s0:s1] if odd else ps[64:128, s0:s1]
                        nc.tensor.matmul(pa, la, ra, start=(t == 4), stop=False)
                        nc.tensor.matmul(pb, lb, rb, start=(t == 4), stop=False)

                    ida = wts[0:64, 576:640]
                    idb = wts[64:128, 576:640]
                    pa = ps[64:128, :] if odd else ps[0:64, :]
                    pb = ps[0:64, :] if odd else ps[64:128, :]
                    nc.tensor.matmul(pa, ida, xb[0:64], start=False, stop=True)
                    nc.tensor.matmul(pb, idb, xb[64:128], start=False, stop=True)

                    tt = work.tile([128, NI], DT.float32, tag="tt")
                    yy = work.tile([128, NI], DT.float32, tag="yy")
                    ps_i = ps.rearrange("p (r c) -> p r c", c=WP)[:, :, 1:1 + W]
                    tt_v = tt.rearrange("p (r c) -> p r c", c=W)
                    nc.scalar.activation(out=tt_v, in_=ps_i, func=AF.Identity,
                                         bias=c_ap, scale=1.0)
                    if fast_prelu:
                        # y = max(t, a*t); valid since d == 0 and a <= 1
                        nc.vector.scalar_tensor_tensor(
                            out=yy, in0=tt, scalar=a_ap, in1=tt,
                            op0=ALU.mult, op1=ALU.max)
                    else:
                        # y = max(t, 0) + d  +  a * min(t, 0); valid for any a, d
                        vv = work.tile([128, NI], DT.float32, tag="vv")
                        nc.vector.tensor_scalar(vv, tt, 0.0, a_ap,
                                                op0=ALU.min, op1=ALU.mult)
                        nc.vector.tensor_scalar(tt, tt, 0.0, d_ap,
                                                op0=ALU.max, op1=ALU.add)
                        nc.vector.tensor_add(yy, tt, vv)

                    y_v = yy.rearrange("p (r c) -> p r c", c=W)
                    lo_img, hi_img = (ib, ia) if odd else (ia, ib)
                    nc.sync.dma_start(out=og[lo_img, :, h0:h0 + RB, :], in_=y_v[0:64])
                    nc.sync.dma_start(out=og[hi_img, :, h0:h0 + RB, :], in_=y_v[64:128])

            emit_loads(0)
            for pr in range(NPAIR):
                if pr + 1 < NPAIR:
                    emit_loads(pr + 1)
                emit_compute(pr)

    nc.compile()
    return nc


def _get_nc(fast_prelu: bool):
    if fast_prelu not in _NC_CACHE:
        _NC_CACHE[fast_prelu] = _build(fast_prelu)
    return _NC_CACHE[fast_prelu]


def _prepare(x, bias0, w, gamma, beta, run_mean, run_var, bias1, alpha, bias2):
    bf16 = DT.np(DT.bfloat16)
    x = np.ascontiguousarray(np.asarray(x, np.float32))
    w = np.asarray(w, np.float32)
    sw = np.sign(w)                                   # [P, C, 3, 3]
    scale = np.abs(w).mean(axis=(1, 2, 3))            # [P]
    inv = np.asarray(gamma, np.float32) / np.sqrt(
        np.asarray(run_var, np.float32) + np.float32(BN_EPS))
    A = (scale * inv).astype(np.float32)
    b1 = np.asarray(bias1, np.float32).reshape(-1)
    b2 = np.asarray(bias2, np.float32).reshape(-1)
    al = np.asarray(alpha, np.float32).reshape(-1)
    b0 = np.asarray(bias0, np.float32).reshape(-1)
    Cc = (np.asarray(beta, np.float32) -
          np.asarray(run_mean, np.float32) * inv + b1 + b2).astype(np.float32)
    dd = (b2 * (1.0 - al)).astype(np.float32)

    wt = np.zeros((128, 640), np.float32)
    for t in range(9):
        blk = (sw[:, :, t // 3, t % 3] * A[:, None]).T      # [C, P]
        wt[0:64, t * 64:(t + 1) * 64] = blk
        wt[64:128, t * 64:(t + 1) * 64] = blk
    ident = np.eye(64, dtype=np.float32)
    wt[0:64, 576:640] = ident
    wt[64:128, 576:640] = ident
    wt_bf = np.ascontiguousarray(wt.astype(bf16))

    cst = np.zeros((128, 8), np.float32)
    for half in range(2):
        sl = slice(half * 64, half * 64 + 64)
        cst[sl, 0] = Cc
        cst[sl, 1] = al
        cst[sl, 2] = dd
        cst[sl, 3] = b0

    fast_prelu = bool(np.all(dd == 0.0) and np.all(al <= 1.0))
    in_maps = [
        {"x": np.ascontiguousarray(x[c * BPC:(c + 1) * BPC]),
         "wts": wt_bf, "cst": cst}
        for c in range(NCORES)
    ]
    return in_maps, fast_prelu


def _run(inputs: dict, trace: bool = False, **spmd_kwargs):
    in_maps, fast_prelu = _prepare(**inputs)
    nc = _get_nc(fast_prelu)
    res = run_bass_kernel_spmd(nc, in_maps, list(range(NCORES)),
                               trace=trace, **spmd_kwargs)
    out = np.concatenate([res.results[c]["out"] for c in range(NCORES)], axis=0)
    return out, res


def kernel(**inputs) -> np.ndarray:
    out, _ = _run(inputs, trace=False)
    return out


# revision 10
# speedup vs baseline: 20.1775x; 20.1775x over previous
"""Trainium2 Bass kernel: binarized (XNOR/ReActNet-style) ResNet BasicBlock.

Computes, for x:[64,64,56,56] f32 and small per-channel parameters:

    out = PReLU_a(BN(conv3x3(sign(x + b0), scale * sign(w))) + x + b1) + b2

Distribution: data-parallel over the batch dim, 8 images per NeuronCore on
8 cores.  Per core, images (i, i+4) share the SBUF partition dim: channels
of the first image on partitions 0-63, channels of the second on 64-127.

Math folding (host side, all tiny tensors):
  - binarized weights sign(w) are pre-scaled by A_m = mean|w|_m * gamma_m /
    sqrt(var_m + eps)  (the BN multiplier), so PSUM holds BN-scaled conv.
    Products are +-A_m exactly, accumulated in fp32 PSUM -> only error is
    bf16 rounding of A_m itself (~2^-9 relative).
  - C_m = beta - mean*inv + bias1 + bias2 is applied as the ScalarE
    activation bias while reading PSUM.
  - residual +x is accumulated into PSUM with an identity matmul (bf16 x).
  - PReLU + bias2: y = max(t, a*t + d), d = bias2*(1-a), valid for a<=1;
    one fused DVE scalar_tensor_tensor when d==0, general 3-op path else.

On-chip layout: activations live in zero-padded 58x58 bf16 planes so each
3x3 tap is one contiguous 464-element matmul rhs slice; x and y live in
unpadded planes so HBM DMAs are 64 descriptors x 12.5KB contiguous.
Conv runs as 9+1 small matmuls per 8-row slice on 2x2 PE quadrants
(tile_position from partition bases); even/odd slices use complementary
quadrant pairs so four matmul streams run concurrently.
"""

import sys

if "/opt/trn_rl_repo" not in sys.path:
    sys.path.insert(0, "/opt/trn_rl_repo")

import numpy as np

import concourse.bass as bass
import concourse.bacc as bacc
import concourse.mybir as mybir
from concourse.tile import TileContext
from concourse.bass_utils import run_bass_kernel_spmd

AF = mybir.ActivationFunctionType
ALU = mybir.AluOpType
DT = mybir.dt

B, C, H, W = 64, 64, 56, 56
NCORES = 8
BPC = B // NCORES          # images per core
NPAIR = BPC // 2           # image pairs per core
HP, WP = H + 2, W + 2      # zero-padded plane 58x58
IMG = HP * WP              # 3364 elements per padded plane
PLN = H * W                # 3136 elements per unpadded plane
RB = 8                     # output rows per slice
NSL = H // RB              # 7 slices per image
NT = RB * WP               # 464: matmul free size (contiguous in padded space)
NI = RB * W                # 448: interior (valid) elements per slice
BN_EPS = 1e-5

_NC_CACHE = {}


def _build(fast_prelu: bool, reps: int = 1):
    nc = bacc.Bacc("TRN2", target_bir_lowering=False, debug=False)
    x_ext = nc.declare_dram_parameter("x", [BPC, C, H, W], DT.float32, isOutput=False)
    w_ext = nc.declare_dram_parameter("wts", [128, 10 * 64], DT.bfloat16, isOutput=False)
    c_ext = nc.declare_dram_parameter("cst", [128, 8], DT.float32, isOutput=False)
    o_ext = nc.declare_dram_parameter("out", [BPC, C, H, W], DT.float32, isOutput=True)

    xg = x_ext.ap().rearrange("b c h w -> b c (h w)")
    og = o_ext.ap().rearrange("b c h w -> b c (h w)")

    with TileContext(nc) as tc:
        with tc.tile_pool(name="persist", bufs=1) as perst, \
             tc.tile_pool(name="work", bufs=4) as work, \
             tc.tile_pool(name="psum", bufs=4, space="PSUM") as ppool:

            wts = perst.tile([128, 10 * 64], DT.bfloat16)
            nc.sync.dma_start(out=wts, in_=w_ext.ap())
            cst = perst.tile([128, 8], DT.float32)
            nc.sync.dma_start(out=cst, in_=c_ext.ap())
            c_ap = cst[:, 0:1]    # beta - mean*inv + bias1 + bias2
            a_ap = cst[:, 1:2]    # PReLU alpha
            d_ap = cst[:, 2:3]    # bias2 * (1 - alpha)
            b0_ap = cst[:, 3:4]   # bias0

            xf = perst.tile([128, NPAIR * PLN], DT.float32)    # unpadded planes
            yb = perst.tile([128, NPAIR * PLN], DT.float32)    # unpadded output
            act = perst.tile([128, NPAIR * IMG], DT.bfloat16)  # padded sign planes

            # residual staging: padded bf16 rows, pad columns zeroed once
            xbs = []
            for i in range(2):
                xb = perst.tile([128, NT], DT.bfloat16, name=f"xb{i}")
                v = xb.rearrange("p (r c) -> p r c", c=WP)
                nc.vector.memset(v[:, :, 0:1], 0.0)
                nc.vector.memset(v[:, :, WP - 1:WP], 0.0)
                xbs.append(xb)

            # zero the padding ring of every act plane
            for pr in range(NPAIR):
                v = act[:, pr * IMG:(pr + 1) * IMG].rearrange(
                    "p (h w) -> p h w", w=WP)
                nc.vector.memset(v[:, 0:1, :], 0.0)
                nc.vector.memset(v[:, HP - 1:HP, :], 0.0)
                nc.vector.memset(v[:, 1:HP - 1, 0:1], 0.0)
                nc.vector.memset(v[:, 1:HP - 1, WP - 1:WP], 0.0)

            def emit_loads(pr):
                ub = pr * PLN
                ab = pr * IMG
                nc.sync.dma_start(out=xf[0:64, ub:ub + PLN], in_=xg[pr])
                nc.sync.dma_start(out=xf[64:128, ub:ub + PLN], in_=xg[pr + NPAIR])
                for rc in range(NSL):
                    h0 = rc * RB
                    src = xf[:, ub + h0 * W:ub + h0 * W + NI].rearrange(
                        "p (r c) -> p r c", c=W)
                    adst = act[:, ab:ab + IMG].rearrange(
                        "p (h w) -> p h w", w=WP)[:, 1 + h0:1 + h0 + RB, 1:1 + W]
                    nc.scalar.activation(out=adst, in_=src, func=AF.Sign,
                                         bias=b0_ap, scale=1.0)

            def emit_xb_copy(s):
                pr, rc = divmod(s, NSL)
                h0 = rc * RB
                xb = xbs[s % 2]
                xb_i = xb.rearrange("p (r c) -> p r c", c=WP)[:, :, 1:1 + W]
                u0 = pr * PLN + h0 * W
                src = xf[:, u0:u0 + NI].rearrange("p (r c) -> p r c", c=W)
                nc.vector.tensor_copy(out=xb_i, in_=src)
                return xb

            def mm_args(s, t):
                pr, rc = divmod(s, NSL)
                h0 = rc * RB
                odd = s % 2
                if t == 9:      # identity (residual) step
                    la = wts[0:64, 576:640]
                    lb = wts[64:128, 576:640]
                    xb = xbs[odd]
                    ra, rb, s0, s1 = xb[0:64], xb[64:128], 0, NT
                else:
                    dh, dw = t // 3 - 1, t % 3 - 1
                    off = pr * IMG + (h0 + dh + 1) * WP + dw
                    s0 = 1 if (rc == 0 and dh == -1 and dw == -1) else 0
                    s1 = NT - 1 if (rc == NSL - 1 and dh == 1 and dw == 1) else NT
                    la = wts[0:64, t * 64:(t + 1) * 64]
                    lb = wts[64:128, t * 64:(t + 1) * 64]
                    ra = act[0:64, off + s0:off + s1]
                    rb = act[64:128, off + s0:off + s1]
                return la, lb, ra, rb, s0, s1, odd

            def emit_mms(slice_group):
                # interleave matmuls of an even+odd slice pair so all four
                # PE quadrants stream concurrently (starts are pc-monotone;
                # disjoint tile_positions overlap)
                pss = {}
                for s in slice_group:
                    pss[s] = ppool.tile([128, NT], DT.float32, tag="ps",
                                        name=f"ps{s}")
                # center tap first: never range-trimmed, so start=True clears
                # the whole bank before the trimmed corner taps accumulate
                for t in (4, 0, 1, 2, 3, 5, 6, 7, 8, 9):
                    for s in slice_group:
                        la, lb, ra, rb, s0, s1, odd = mm_args(s, t)
                        ps = pss[s]
                        pa = ps[64:128, s0:s1] if odd else ps[0:64, s0:s1]
                        pb = ps[0:64, s0:s1] if odd else ps[64:128, s0:s1]
                        nc.tensor.matmul(pa, la, ra, start=(t == 4),
                                         stop=(t == 9), skip_group_check=True)
                        nc.tensor.matmul(pb, lb, rb, start=(t == 4),
                                         stop=(t == 9), skip_group_check=True)
                return pss

            def emit_epilogue(s, ps):
                pr, rc = divmod(s, NSL)
                h0 = rc * RB
                u0 = pr * PLN + h0 * W
                tt = work.tile([128, NI], DT.float32, tag="tt")
                ps_i = ps.rearrange("p (r c) -> p r c", c=WP)[:, :, 1:1 + W]
                tt_v = tt.rearrange("p (r c) -> p r c", c=W)
                nc.scalar.activation(out=tt_v, in_=ps_i, func=AF.Identity,
                                     bias=c_ap, scale=1.0)
                yv = yb[:, u0:u0 + NI]
                if fast_prelu:
                    # y = max(t, a*t); valid since d == 0 and a <= 1
                    nc.vector.scalar_tensor_tensor(
                        out=yv, in0=tt, scalar=a_ap, in1=tt,
                        op0=ALU.mult, op1=ALU.max)
                else:
                    # y = max(t, 0) + d  +  a * min(t, 0); any a, d
                    vv = work.tile([128, NI], DT.float32, tag="vv")
                    nc.vector.tensor_scalar(vv, tt, 0.0, a_ap,
                                            op0=ALU.min, op1=ALU.mult)
                    nc.vector.tensor_scalar(tt, tt, 0.0, d_ap,
                                            op0=ALU.max, op1=ALU.add)
                    nc.vector.tensor_add(yv, tt, vv)

            def emit_store(pr):
                # output store, split by slice parity: odd-parity slices have
                # swapped halves (image B on partitions 0-63) from the crossed
                # PE quadrants, so route each parity's chunks separately
                ub = pr * PLN
                ia, ib = pr, pr + NPAIR
                ov_a = og[ia].rearrange("c (k n) -> c k n", n=NI)
                ov_b = og[ib].rearrange("c (k n) -> c k n", n=NI)
                yv_lo = yb[0:64, ub:ub + PLN].rearrange("c (k n) -> c k n", n=NI)
                yv_hi = yb[64:128, ub:ub + PLN].rearrange("c (k n) -> c k n", n=NI)
                pn = pr % 2          # rc parity whose layout is normal [A|B]
                psw = 1 - pn
                nc.sync.dma_start(out=ov_a[:, pn:NSL:2], in_=yv_lo[:, pn:NSL:2])
                nc.sync.dma_start(out=ov_b[:, pn:NSL:2], in_=yv_hi[:, pn:NSL:2])
                nc.sync.dma_start(out=ov_a[:, psw:NSL:2], in_=yv_hi[:, psw:NSL:2])
                nc.sync.dma_start(out=ov_b[:, psw:NSL:2], in_=yv_lo[:, psw:NSL:2])

            def emit_compute_all():
                nsl_tot = NPAIR * NSL
                loaded = 1
                s = 0
                while s < nsl_tot:
                    group = [s] if s + 1 >= nsl_tot else [s, s + 1]
                    # keep x loads one image-pair ahead of compute
                    need_pr = (group[-1] // NSL) + 1
                    while loaded <= need_pr and loaded < NPAIR:
                        emit_loads(loaded)
                        loaded += 1
                    for g in group:
                        emit_xb_copy(g)
                    pss = emit_mms(group)
                    for g in group:
                        emit_epilogue(g, pss[g])
                    for g in group:
                        if (g + 1) % NSL == 0:
                            emit_store(g // NSL)
                    s += len(group)

            for _ in range(reps):
                emit_loads(0)
                emit_compute_all()

    nc.compile()
    return nc


def _get_nc(fast_prelu: bool, reps: int = 1):
    key = (fast_prelu, reps)
    if key not in _NC_CACHE:
        _NC_CACHE[key] = _build(fast_prelu, reps)
    return _NC_CACHE[key]


def _prepare(x, bias0, w, gamma, beta, run_mean, run_var, bias1, alpha, bias2):
    bf16 = DT.np(DT.bfloat16)
    x = np.ascontiguousarray(np.asarray(x, np.float32))
    w = np.asarray(w, np.float32)
    sw = np.sign(w)                                   # [P, C, 3, 3]
    scale = np.abs(w).mean(axis=(1, 2, 3))            # [P]
    inv = np.asarray(gamma, np.float32) / np.sqrt(
        np.asarray(run_var, np.float32) + np.float32(BN_EPS))
    A = (scale * inv).astype(np.float32)
    b1 = np.asarray(bias1, np.float32).reshape(-1)
    b2 = np.asarray(bias2, np.float32).reshape(-1)
    al = np.asarray(alpha, np.float32).reshape(-1)
    b0 = np.asarray(bias0, np.float32).reshape(-1)
    Cc = (np.asarray(beta, np.float32) -
          np.asarray(run_mean, np.float32) * inv + b1 + b2).astype(np.float32)
    dd = (b2 * (1.0 - al)).astype(np.float32)

    wt = np.zeros((128, 640), np.float32)
    for t in range(9):
        blk = (sw[:, :, t // 3, t % 3] * A[:, None]).T      # [C, P]
        wt[0:64, t * 64:(t + 1) * 64] = blk
        wt[64:128, t * 64:(t + 1) * 64] = blk
    ident = np.eye(64, dtype=np.float32)
    wt[0:64, 576:640] = ident
    wt[64:128, 576:640] = ident
    wt_bf = np.ascontiguousarray(wt.astype(bf16))

    cst = np.zeros((128, 8), np.float32)
    for half in range(2):
        sl = slice(half * 64, half * 64 + 64)
        cst[sl, 0] = Cc
        cst[sl, 1] = al
        cst[sl, 2] = dd
        cst[sl, 3] = b0

    fast_prelu = bool(np.all(dd == 0.0) and np.all(al <= 1.0))
    in_maps = [
        {"x": np.ascontiguousarray(x[c * BPC:(c + 1) * BPC]),
         "wts": wt_bf, "cst": cst}
        for c in range(NCORES)
    ]
    return in_maps, fast_prelu


_RUNNER_CACHE = {}


def _make_runner(nc, n_cores=NCORES):
    """Build a reusable jitted executor for `nc` (one XLA trace, NEFF cached)."""
    import jax
    from jax.sharding import Mesh, PartitionSpec, NamedSharding
    from jax.experimental.shard_map import shard_map
    from concourse import bass2jax

    bass2jax.install_neuronx_cc_hook()
    partition_name = nc.partition_id_tensor.name if nc.partition_id_tensor else None
    in_names, out_names, out_avals, zero_outs = [], [], [], []
    for alloc in nc.m.functions[0].allocations:
        if not isinstance(alloc, mybir.MemoryLocationSet):
            continue
        name = alloc.memorylocations[0].name
        if alloc.kind == "ExternalInput":
            if name != partition_name:
                in_names.append(name)
        elif alloc.kind == "ExternalOutput":
            out_names.append(name)
            shape = tuple(alloc.tensor_shape)
            dtype = mybir.dt.np(alloc.dtype)
            out_avals.append(jax.core.ShapedArray(shape, dtype))
            zero_outs.append(np.zeros(shape, dtype))
    n_params = len(in_names)
    all_in = list(in_names) + out_names + ([partition_name] if partition_name else [])

    def _body(*args):
        operands = list(args)
        if partition_name is not None:
            operands.append(bass2jax.partition_id_tensor())
        outs = bass2jax._bass_exec_p.bind(
            *operands,
            out_avals=tuple(out_avals),
            in_names=tuple(all_in),
            out_names=tuple(out_names),
            lowering_input_output_aliases=(),
            sim_require_finite=True,
            sim_require_nnan=True,
            nc=nc,
        )
        return tuple(outs)

    devices = jax.devices()[:n_cores]
    mesh = Mesh(np.asarray(devices), ("core",))
    nin = n_params + len(out_names)
    f = jax.jit(shard_map(
        _body, mesh=mesh,
        in_specs=(PartitionSpec("core"),) * nin,
        out_specs=(PartitionSpec("core"),) * len(out_names),
        check_rep=False))
    sh = NamedSharding(mesh, PartitionSpec("core"))
    concat_zeros = [
        jax.device_put(np.zeros((n_cores * z.shape[0], *z.shape[1:]), z.dtype), sh)
        for z in zero_outs
    ]

    def run(in_maps):
        concat_in = [
            np.concatenate([np.asarray(in_maps[c][nm]) for c in range(n_cores)],
                           axis=0)
            for nm in in_names
        ]
        args = [jax.device_put(a, sh) for a in concat_in] + concat_zeros
        outs = f(*args)
        jax.block_until_ready(outs)
        oi = out_names.index("out")
        full = np.asarray(outs[oi])
        return full.reshape(n_cores, *out_avals[oi].shape)

    run.jit_fn = f
    run.sharding = sh
    run.in_names = in_names
    run.out_names = out_names
    run.zero_args = concat_zeros
    return run


def _get_runner(fast_prelu: bool, reps: int = 1):
    key = (fast_prelu, reps)
    if key not in _RUNNER_CACHE:
        _RUNNER_CACHE[key] = _make_runner(_get_nc(fast_prelu, reps))
    return _RUNNER_CACHE[key]


def _run(inputs: dict, trace: bool = False, reps: int = 1, **spmd_kwargs):
    """Legacy path through run_bass_kernel_spmd (used for debugging)."""
    in_maps, fast_prelu = _prepare(**inputs)
    nc = _get_nc(fast_prelu, reps)
    res = run_bass_kernel_spmd(nc, in_maps, list(range(NCORES)),
                               trace=trace, **spmd_kwargs)
    out = np.concatenate([res.results[c]["out"] for c in range(NCORES)], axis=0)
    return out, res


def kernel(**inputs) -> np.ndarray:
    in_maps, fast_prelu = _prepare(**inputs)
    runner = _get_runner(fast_prelu)
    per_core = runner(in_maps)
    return np.ascontiguousarray(per_core.reshape(B, C, H, W))
